# revision 16
# baseline (speedup 1.0000x reference)
"""Trainium2 Bass kernel for nn_Attention_Weighted_Context_Generation.

Computes ctx = A @ F where
  A = weights.reshape(9216, 9216)              (row i = output location)
  F = cnn_feature.reshape(256, 9216).T          [9216, 256]
returns ctx.reshape(9216, 1, 1, 256) float32.

Sharding: rows of A (the HW/location dim) split across 8 NeuronCores,
1152 rows each; F replicated. Each core's shard is packed host-side as
one contiguous [9216, 1152+256] array whose row j holds
[A[m0:m1, j] , F[j, :]] — i.e. the A-shard transposed (so the
contraction dim lands on SBUF partitions with unit-stride DMA) with the
matching F rows appended. The kernel streams 72 k-tiles of [128, 1408]
and accumulates 9 PSUM slices of [128m, 256c] across the whole k range.
"""

import numpy as np

import concourse.bass as bass
from concourse import mybir
from concourse.bass_utils import run_bass_kernel_spmd

N_CORES = 8
HW = 9216            # number of locations = 96*96
C = 256              # channels
M_PER = HW // N_CORES  # 1152 output rows per core
KT = HW // 128         # 72 contraction tiles
MT = M_PER // 128      # 9 output row-tiles per core
W_COLS = M_PER + C     # 1408 packed columns
import os
NBUF = int(os.environ.get("K_NBUF", "6"))   # SBUF ring depth for streamed k-tiles
DMA_LAG = int(os.environ.get("K_LAG", "1"))  # extra completed DMAs required
                                             # before consuming tile jt
PE_NOP = int(os.environ.get("K_PE_NOP", "0"))  # debug: stall PE per group
DVE_COLS = 6 * C       # PSUM evacuation split at a bank boundary (1536 f32)

# PE dtype: float32 is exact but 4 cyc/row (PE-bound ~305us); float32r is
# 1 cyc/row but numerically unstable on HW; bfloat16 is 1 cyc/row with
# ~3e-3 rel err. With bfloat16 the k-tiles are cast fp32->bf16 in-flight
# by the SWDGE DMA path (HBM still reads the full fp32 bytes).
COMPUTE_DT = mybir.dt.float32r


def build_bass():
    nc = bass.Bass("TRN2", target_bir_lowering=False, debug=False,
                   num_devices=N_CORES)
    # float32r is bit-identical to float32; declaring the DRAM input as
    # f32r avoids a pointless dtype "cast" in the load DMA.
    atf_dt = (mybir.dt.float32r if COMPUTE_DT == mybir.dt.float32r
              else mybir.dt.float32)
    atf = nc.dram_tensor("atf", [HW, W_COLS], atf_dt,
                         kind="ExternalInput").ap()
    out = nc.dram_tensor("out", [M_PER, C], mybir.dt.float32,
                         kind="ExternalOutput").ap()

    from contextlib import ExitStack
    with (
        ExitStack() as stack,
        nc.sbuf_tensor("kbufs", [128, NBUF * W_COLS], COMPUTE_DT) as kbufs,
        nc.sbuf_tensor("out_sb", [128, MT * C], mybir.dt.float32) as out_sb,
        nc.psum_tensor("acc", [128, MT * C], mybir.dt.float32) as acc,
        nc.semaphore("mm_sem") as mm_sem,
        nc.semaphore("evac_sem") as evac_sem,
        nc.semaphore("out_sem") as out_sem,
        nc.Block() as block,
    ):
        # DMA-completion sems rotate: with a single shared sem, the 16
        # per-SDMA-engine increments of consecutive DMAs interleave, so
        # "sem >= 16*(jt+1)" does NOT imply DMA jt's data landed (measured
        # ~850ns sem-lead in NTFF traces -> stale-tile matmuls). Per-engine
        # descriptor FIFO makes a rotation of NSEM sems safe against up to
        # NSEM-1 DMAs of cross-engine skew.
        NSEM = 8
        dma_sems = [stack.enter_context(nc.semaphore(f"dma_sem{i}"))
                    for i in range(NSEM)]

        # fp32->bf16 cast-in-flight requires the SWDGE (gpsimd) DMA path;
        # plain fp32/f32r loads go on the faster HWDGE (sync) path.
        cast_loads = COMPUTE_DT not in (mybir.dt.float32, mybir.dt.float32r)

        def emit_loads(eng):
            for jt in range(KT):
                if jt >= NBUF:
                    # ring slot reused: wait until its matmuls retired
                    eng.wait_ge(mm_sem, jt - NBUF + 1)
                b = jt % NBUF
                eng.dma_start(
                    out=kbufs[:, b * W_COLS:(b + 1) * W_COLS],
                    in_=atf[jt * 128:(jt + 1) * 128, :],
                ).then_inc(dma_sems[jt % NSEM], 16)

        if cast_loads:
            @block.gpsimd
            def _(gpsimd):
                emit_loads(gpsimd)

        @block.sync
        def _(sync):
            if not cast_loads:
                emit_loads(sync)
            sync.wait_ge(evac_sem, 2)
            sync.dma_start(
                out=out.rearrange("(a p) c -> p a c", p=128),
                in_=out_sb[:].rearrange("p (a c) -> p a c", a=MT),
            ).then_inc(out_sem, 16)
            sync.wait_ge(out_sem, 16)

        @block.tensor
        def _(tensor):
            for jt in range(KT):
                tensor.wait_ge(dma_sems[jt % NSEM], 16 * (jt // NSEM + 1))
                if PE_NOP:
                    tensor.nop(cycle_cnt=PE_NOP, nofuse=True)
                b = jt % NBUF
                buf = kbufs[:, b * W_COLS:(b + 1) * W_COLS]
                f_tile = buf[:, M_PER:W_COLS]
                inst = None
                for mi in range(MT):
                    # Two 256-f32 chains share each 512-f32 PSUM bank, and
                    # start=True clears has_written for the WHOLE bank. Only
                    # the bank's first chain (even mi) may clear; the odd
                    # chain's first matmul relies on its bits being clear
                    # already (overwrite-and-set, no bank clear).
                    inst = tensor.matmul(
                        acc[:, mi * C:(mi + 1) * C],
                        buf[:, mi * 128:(mi + 1) * 128],
                        f_tile,
                        start=(jt == 0 and mi % 2 == 0),
                        stop=(jt == KT - 1),
                    )
                inst.then_inc(mm_sem, 1)

        @block.vector
        def _(vector):
            vector.wait_ge(mm_sem, KT)
            vector.tensor_copy(out_sb[:, :DVE_COLS],
                               acc[:, :DVE_COLS]).then_inc(evac_sem, 1)

        @block.scalar
        def _(scalar):
            scalar.wait_ge(mm_sem, KT)
            scalar.copy(out_sb[:, DVE_COLS:],
                        acc[:, DVE_COLS:]).then_inc(evac_sem, 1)

    return nc


def prep_inputs(weights: np.ndarray, cnn_feature: np.ndarray):
    """Pack per-core [9216, 1408] float32 arrays: [A_shard^T | F]."""
    A = np.ascontiguousarray(weights.reshape(HW, HW))
    F = np.ascontiguousarray(cnn_feature.reshape(C, HW).T)  # [HW, C]
    in_maps = []
    for i in range(N_CORES):
        at = A[i * M_PER:(i + 1) * M_PER, :].T  # [HW, M_PER] view
        atf = np.concatenate([at, F], axis=1)   # [HW, 1408] contiguous
        in_maps.append({"atf": atf})
    return in_maps


def kernel(weights: np.ndarray, cnn_feature: np.ndarray) -> np.ndarray:
    in_maps = prep_inputs(np.asarray(weights), np.asarray(cnn_feature))
    nc = build_bass()
    res = run_bass_kernel_spmd(nc, in_maps, list(range(N_CORES)))
    ctx = np.concatenate([res.results[i]["out"] for i in range(N_CORES)],
                         axis=0)
    return ctx.reshape(HW, 1, 1, C).astype(np.float32, copy=False)
